# revision 2
# baseline (speedup 1.0000x reference)
"""DeepseekV2 MoE layer on 8 Trainium2 NeuronCores (Bass/Tile).

Strategy (expert-parallel, per sharding hint):
  - 16 routed experts sharded 2-per-core; shared-expert intermediate dim
    (2816) sharded 8-way (tensor-parallel Column/RowParallel style).
  - Router (gate) replicated: each core computes softmax-free top-6 combine
    weights for all 1024 tokens in fp32.
  - Expert MLPs run dense over all tokens in float32r (full PE rate,
    ~1.5e-4 rel err). Combine weights are pre-multiplied into the SwiGLU
    activations, so each core's two routed experts AND its shared-expert
    slice accumulate into a single PSUM group in the down-projection.
  - Per-core partial [1024, 2048] outputs are summed with two
    ReduceScatter collectives (one per 512-token half, overlapping the
    second half's compute); host reassembles the 8 shards.

Weights are pre-transposed host-side into contraction-major layouts
(h-major for gate/up, i-major for down) because the TensorE contracts
along the SBUF partition dim and transposed-AP DMA loads are ~19x slower.
"""

import numpy as np

import concourse.bass as bass
import concourse.mybir as mybir
import concourse.tile as tile
from concourse import bacc
from concourse import bass_utils
from concourse.bass_interp import get_hw_module
from concourse.masks import make_identity

F32 = mybir.dt.float32
F32R = mybir.dt.float32r
AX = mybir.AxisListType
ALU = mybir.AluOpType
ACTF = mybir.ActivationFunctionType

T = 1024      # tokens
H = 2048      # hidden
I = 1408      # moe intermediate
E = 16        # routed experts
K = 6         # experts per token
SI = 2816     # shared intermediate
NC = 8        # cores
EPC = E // NC            # experts per core (2)
SIL = SI // NC           # shared intermediate per core (352)
TH = 512                 # token half
NIT = I // 128           # routed i-tiles (11)
SH_TILES = [128, 128, 96]  # 352 = 128+128+96
HB = 256                 # stage-B h block
NHB = H // HB            # 8 h blocks


def _build_program():
    nc = bacc.Bacc("TRN2", target_bir_lowering=False, debug=False,
                   enable_asserts=False, num_devices=NC)

    # ---- DRAM I/O (per-core shards; f32 bytes, f32r view for matmul paths) ----
    xT_d = nc.dram_tensor("xT", [H, T], F32R, kind="ExternalInput")
    gwT_d = nc.dram_tensor("gwT", [H, E], F32, kind="ExternalInput")
    wgT_d = nc.dram_tensor("wgT", [EPC, H, I], F32R, kind="ExternalInput")
    wuT_d = nc.dram_tensor("wuT", [EPC, H, I], F32R, kind="ExternalInput")
    wdT_d = nc.dram_tensor("wdT", [EPC, I, H], F32R, kind="ExternalInput")
    swgT_d = nc.dram_tensor("swgT", [H, SIL], F32R, kind="ExternalInput")
    swuT_d = nc.dram_tensor("swuT", [H, SIL], F32R, kind="ExternalInput")
    swdT_d = nc.dram_tensor("swdT", [SIL, H], F32R, kind="ExternalInput")
    esel_d = nc.dram_tensor("esel", [E, EPC * 128], F32, kind="ExternalInput")
    out_d = nc.dram_tensor("out", [2 * (TH // NC), H], F32, kind="ExternalOutput")

    with tile.TileContext(nc) as tc:
        with (
            tc.tile_pool(name="const", bufs=1) as cpool,
            tc.tile_pool(name="combT", bufs=1) as combT_pool,
            tc.tile_pool(name="cbc", bufs=1) as cbc_pool,
            tc.tile_pool(name="xtr", bufs=1) as xtr_pool,
            tc.tile_pool(name="ch", bufs=1) as ch_pool,
            tc.tile_pool(name="wgu", bufs=2) as wgu_pool,
            tc.tile_pool(name="wd", bufs=2) as wd_pool,
            tc.tile_pool(name="act", bufs=2) as act_pool,
            tc.tile_pool(name="small", bufs=2) as sm_pool,
            tc.tile_pool(name="ob", bufs=3) as ob_pool,
            tc.tile_pool(name="psr", bufs=2, space="PSUM") as psr_pool,
            tc.tile_pool(name="psa", bufs=2, space="PSUM") as psa_pool,
            tc.tile_pool(name="psb", bufs=2, space="PSUM") as psb_pool,
            tc.tile_pool(name="dram", bufs=1, space="DRAM") as dram_pool,
        ):
            # ---- constants ----
            ident = cpool.tile([128, 128], F32)
            make_identity(nc, ident[:])
            gwT_sb = cpool.tile([128, H // 128, E], F32)
            nc.sync.dma_start(
                gwT_sb[:], gwT_d[:].rearrange("(c p) e -> p c e", p=128))
            esel_sb = cpool.tile([E, EPC * 128], F32)
            nc.sync.dma_start(esel_sb[:], esel_d[:])

            ccin = [dram_pool.tile([TH, H], F32, name=f"ccin{h}") for h in (0, 1)]
            ccout = [dram_pool.tile([TH // NC, H], F32, name=f"ccout{h}")
                     for h in (0, 1)]

            for half in (0, 1):
                t0 = half * TH

                # ---- x^T (f32r) for this half: [128, 16 h-chunks, TH] ----
                xTr = xtr_pool.tile([128, H // 128, TH], F32R, tag="xTr")
                for hc in range(H // 128):
                    nc.sync.dma_start(
                        xTr[:, hc, :],
                        xT_d[hc * 128:(hc + 1) * 128, t0:t0 + TH])

                # ---- router: logits -> top6 combine weights (f32) ----
                combT = combT_pool.tile([E, TH], F32, tag="combT")
                for tt in range(TH // 128):
                    psl = psr_pool.tile([128, E], F32, tag="psr")
                    for hc in range(H // 128):
                        nc.tensor.matmul(
                            psl[:],
                            xTr[:, hc, tt * 128:(tt + 1) * 128].bitcast(F32),
                            gwT_sb[:, hc, :],
                            start=(hc == 0), stop=(hc == H // 128 - 1))
                    mx = sm_pool.tile([128, 1], F32, tag="mx")
                    nc.vector.reduce_max(mx[:], psl[:], axis=AX.X)
                    ee = sm_pool.tile([128, E], F32, tag="ee")
                    nc.vector.tensor_scalar(ee[:], psl[:], mx[:], None,
                                            op0=ALU.subtract)
                    nc.scalar.activation(ee[:], ee[:], ACTF.Exp)
                    top8 = sm_pool.tile([128, 8], F32, tag="top8")
                    nc.vector.max(out=top8[:], in_=ee[:])
                    mask = sm_pool.tile([128, E], F32, tag="mask")
                    nc.vector.tensor_scalar(mask[:], ee[:], top8[:, K - 1:K],
                                            None, op0=ALU.is_ge)
                    s6 = sm_pool.tile([128, 1], F32, tag="s6")
                    nc.vector.reduce_sum(s6[:], top8[:, 0:K], axis=AX.X)
                    r6 = sm_pool.tile([128, 1], F32, tag="r6")
                    nc.vector.reciprocal(r6[:], s6[:])
                    num = sm_pool.tile([128, E], F32, tag="num")
                    nc.vector.tensor_mul(num[:], ee[:], mask[:])
                    comb = sm_pool.tile([128, E], F32, tag="comb")
                    nc.vector.tensor_scalar(comb[:], num[:], r6[:], None,
                                            op0=ALU.mult)
                    pst = psr_pool.tile([E, 128], F32, tag="psr")
                    nc.tensor.transpose(pst[:], comb[:], ident[:])
                    nc.scalar.copy(combT[:, tt * 128:(tt + 1) * 128], pst[:])

                # broadcast comb rows of this core's experts to all partitions
                cbc = cbc_pool.tile([128, EPC, TH], F32, tag="cbc")
                for j in range(EPC):
                    pscb = psr_pool.tile([128, TH], F32, tag="psr")
                    nc.tensor.matmul(pscb[:], esel_sb[:, j * 128:(j + 1) * 128],
                                     combT[:], start=True, stop=True)
                    nc.vector.tensor_copy(cbc[:, j, :], pscb[:])

                # ---- stage A: SwiGLU activations (f32r), comb pre-scaled ----
                def stage_a(gT_ap, uT_ap, widths, ch_tag, comb_j):
                    n_h = H // 128
                    i0 = 0
                    for it, w in enumerate(widths):
                        wgc = wgu_pool.tile([128, n_h, 128], F32R, tag="wg")
                        wuc = wgu_pool.tile([128, n_h, 128], F32R, tag="wu")
                        nc.sync.dma_start(
                            wgc[:, :, :w],
                            gT_ap[:, i0:i0 + w].rearrange("(c p) i -> p c i", p=128))
                        nc.sync.dma_start(
                            wuc[:, :, :w],
                            uT_ap[:, i0:i0 + w].rearrange("(c p) i -> p c i", p=128))
                        psg = psa_pool.tile([128, TH], F32, tag="psg")
                        psu = psa_pool.tile([128, TH], F32, tag="psu")
                        for hc in range(n_h):
                            nc.tensor.matmul(psg[:w], wgc[:, hc, :w], xTr[:, hc, :],
                                             start=(hc == 0), stop=(hc == n_h - 1))
                        for hc in range(n_h):
                            nc.tensor.matmul(psu[:w], wuc[:, hc, :w], xTr[:, hc, :],
                                             start=(hc == 0), stop=(hc == n_h - 1))
                        sg = act_pool.tile([128, TH], F32, tag="sg")
                        nc.scalar.activation(sg[:w], psg[:w], ACTF.Silu)
                        ch = ch_pool.tile([128, TH], F32R, tag=f"{ch_tag}_{it}")
                        if comb_j is None:
                            nc.vector.tensor_mul(ch[:w], sg[:w], psu[:w])
                        else:
                            tmp = act_pool.tile([128, TH], F32, tag="tmp")
                            nc.vector.tensor_mul(tmp[:w], sg[:w], psu[:w])
                            nc.vector.tensor_mul(ch[:w], tmp[:w],
                                                 cbc[:, comb_j, :])
                        ch_tiles[ch_tag].append((ch, w))
                        i0 += w

                ch_tiles = {"ch0": [], "ch1": [], "chs": []}
                for j in range(EPC):
                    stage_a(wgT_d[j], wuT_d[j], [128] * NIT, f"ch{j}", j)
                stage_a(swgT_d[:], swuT_d[:], SH_TILES, "chs", None)

                # ---- stage B: fused down-projection, one PSUM group per tile ----
                n_acc = EPC * NIT + len(SH_TILES)
                for hb in range(NHB):
                    h0 = hb * HB
                    wds = []
                    for j in range(EPC):
                        wd = wd_pool.tile([128, NIT, HB], F32R, tag=f"wd{j}")
                        nc.sync.dma_start(
                            wd[:],
                            wdT_d[j][:, h0:h0 + HB].rearrange(
                                "(c p) h -> p c h", p=128))
                        wds.append(wd)
                    wsd = wd_pool.tile([128, len(SH_TILES), HB], F32R, tag="wds")
                    nc.sync.dma_start(
                        wsd[:, 0:2, :],
                        swdT_d[0:256, h0:h0 + HB].rearrange(
                            "(c p) h -> p c h", p=128))
                    nc.sync.dma_start(wsd[:96, 2, :], swdT_d[256:352, h0:h0 + HB])

                    for tt in range(TH // 128):
                        ts_ = slice(tt * 128, (tt + 1) * 128)
                        ps = psb_pool.tile([128, HB], F32, tag="psb")
                        k = 0
                        for j in range(EPC):
                            for it, (ch, w) in enumerate(ch_tiles[f"ch{j}"]):
                                nc.tensor.matmul(ps[:], ch[:w, ts_],
                                                 wds[j][:w, it, :],
                                                 start=(k == 0),
                                                 stop=(k == n_acc - 1))
                                k += 1
                        for it, (ch, w) in enumerate(ch_tiles["chs"]):
                            nc.tensor.matmul(ps[:], ch[:w, ts_], wsd[:w, it, :],
                                             start=(k == 0),
                                             stop=(k == n_acc - 1))
                            k += 1
                        ob = ob_pool.tile([128, HB], F32, tag="ob")
                        nc.scalar.copy(ob[:], ps[:])
                        nc.sync.dma_start(
                            ccin[half][tt * 128:(tt + 1) * 128, h0:h0 + HB],
                            ob[:])

                # ---- sum partials across cores; each core keeps 64 rows ----
                nc.gpsimd.collective_compute(
                    "ReduceScatter",
                    ALU.add,
                    replica_groups=[list(range(NC))],
                    ins=[ccin[half][:].opt()],
                    outs=[ccout[half][:].opt()],
                )
                nc.sync.dma_start(
                    out_d[half * (TH // NC):(half + 1) * (TH // NC), :],
                    ccout[half][:])

    nc.compile()
    nc.m = get_hw_module(nc.m)
    return nc


_PROGRAM = None


def _get_program():
    global _PROGRAM
    if _PROGRAM is None:
        _PROGRAM = _build_program()
    return _PROGRAM


def _prep_in_maps(x, gate_w, w_gate, w_up, w_down, sw_gate, sw_up, sw_down):
    f = np.float32
    xT = np.ascontiguousarray(np.asarray(x, f).T)                  # [H, T]
    gwT = np.ascontiguousarray(np.asarray(gate_w, f).T)            # [H, E]
    wgT = np.ascontiguousarray(np.asarray(w_gate, f).transpose(0, 2, 1))
    wuT = np.ascontiguousarray(np.asarray(w_up, f).transpose(0, 2, 1))
    wdT = np.ascontiguousarray(np.asarray(w_down, f).transpose(0, 2, 1))
    swgT = np.ascontiguousarray(np.asarray(sw_gate, f).T)          # [H, SI]
    swuT = np.ascontiguousarray(np.asarray(sw_up, f).T)
    swdT = np.ascontiguousarray(np.asarray(sw_down, f).T)          # [SI, H]

    in_maps = []
    for r in range(NC):
        esel = np.zeros((E, EPC * 128), f)
        for j in range(EPC):
            esel[EPC * r + j, j * 128:(j + 1) * 128] = 1.0
        in_maps.append({
            "xT": xT,
            "gwT": gwT,
            "wgT": np.ascontiguousarray(wgT[EPC * r:EPC * (r + 1)]),
            "wuT": np.ascontiguousarray(wuT[EPC * r:EPC * (r + 1)]),
            "wdT": np.ascontiguousarray(wdT[EPC * r:EPC * (r + 1)]),
            "swgT": np.ascontiguousarray(swgT[:, SIL * r:SIL * (r + 1)]),
            "swuT": np.ascontiguousarray(swuT[:, SIL * r:SIL * (r + 1)]),
            "swdT": np.ascontiguousarray(swdT[SIL * r:SIL * (r + 1), :]),
            "esel": esel,
        })
    return in_maps


def kernel(x, gate_w, w_gate, w_up, w_down, sw_gate, sw_up, sw_down,
           _trace=False):
    nc = _get_program()
    in_maps = _prep_in_maps(x, gate_w, w_gate, w_up, w_down,
                            sw_gate, sw_up, sw_down)
    res = bass_utils.run_bass_kernel_spmd(
        nc, in_maps, core_ids=list(range(NC)), trace=_trace)

    out = np.empty((T, H), np.float32)
    rows = TH // NC
    for r in range(NC):
        o = res.results[r]["out"]
        out[rows * r:rows * (r + 1)] = o[:rows]
        out[TH + rows * r:TH + rows * (r + 1)] = o[rows:]
    if _trace:
        kernel._last_results = res
    return out


# revision 3
# speedup vs baseline: 1.0364x; 1.0364x over previous
"""DeepseekV2 MoE layer on 8 Trainium2 NeuronCores (Bass/Tile).

Strategy (expert-parallel, per sharding hint):
  - 16 routed experts sharded 2-per-core; shared-expert intermediate dim
    (2816) sharded 8-way (tensor-parallel Column/RowParallel style).
  - Router (gate) replicated: each core computes softmax-free top-6 combine
    weights for all 1024 tokens in fp32 (exact expert selection).
  - Expert MLPs run dense over all tokens in bf16 (f32 PSUM accumulate).
    Combine weights are pre-multiplied into the SwiGLU activations, so each
    core's two routed experts AND its shared-expert slice accumulate into a
    single PSUM group in the down-projection.
  - Per-core partial [1024, 2048] outputs are summed with two
    ReduceScatter collectives (split along hidden dim so the first
    overlaps the second half of the down-projection); host reassembles.

Weights are pre-transposed (contraction-major) and pre-cast to bf16
host-side: TensorE contracts along the SBUF partition dim, transposed-AP
DMA is ~19x slower, and bf16 halves HBM traffic + enables fast weight load.
"""

import numpy as np
import ml_dtypes

import concourse.bass as bass
import concourse.mybir as mybir
import concourse.tile as tile
from concourse import bacc
from concourse import bass_utils
from concourse.bass_interp import get_hw_module
from concourse.masks import make_identity

F32 = mybir.dt.float32
BF16 = mybir.dt.bfloat16
AX = mybir.AxisListType
ALU = mybir.AluOpType
ACTF = mybir.ActivationFunctionType

T = 1024      # tokens
H = 2048      # hidden
I = 1408      # moe intermediate
E = 16        # routed experts
K = 6         # experts per token
SI = 2816     # shared intermediate
NC = 8        # cores
EPC = E // NC            # experts per core (2)
SIL = SI // NC           # shared intermediate per core (352)
NHC = H // 128           # h chunks (16)
TB = 512                 # stage-A token block (psum free dim)
NTB = T // TB            # 2
RT_I = [256] * 5 + [128]   # routed i-tile loads (512B dma runs)
SH_I = [256, 96]           # shared i-tile loads
HB = 256                 # stage-B h block
NHB = H // HB            # 8 h blocks


def _mm_tiles(widths):
    """(load_idx, offset_in_load, global_i0, m_width) per 128-wide matmul."""
    out = []
    g0 = 0
    for li, w in enumerate(widths):
        o = 0
        while o < w:
            m = min(128, w - o)
            out.append((li, o, g0 + o, m))
            o += m
    return out


def _build_program():
    nc = bacc.Bacc("TRN2", target_bir_lowering=False, debug=False,
                   enable_asserts=False, num_devices=NC)

    xT32_d = nc.dram_tensor("xT32", [H, T], F32, kind="ExternalInput")
    xT_d = nc.dram_tensor("xT", [H, T], BF16, kind="ExternalInput")
    gwT_d = nc.dram_tensor("gwT", [H, E], F32, kind="ExternalInput")
    wgT_d = nc.dram_tensor("wgT", [EPC, H, I], BF16, kind="ExternalInput")
    wuT_d = nc.dram_tensor("wuT", [EPC, H, I], BF16, kind="ExternalInput")
    wdT_d = nc.dram_tensor("wdT", [EPC, I, H], BF16, kind="ExternalInput")
    swgT_d = nc.dram_tensor("swgT", [H, SIL], BF16, kind="ExternalInput")
    swuT_d = nc.dram_tensor("swuT", [H, SIL], BF16, kind="ExternalInput")
    swdT_d = nc.dram_tensor("swdT", [SIL, H], BF16, kind="ExternalInput")
    esel_d = nc.dram_tensor("esel", [E, EPC * 128], F32, kind="ExternalInput")
    out_d = nc.dram_tensor("out", [T // NC, H], F32, kind="ExternalOutput")

    with tile.TileContext(nc) as tc:
        with (
            tc.tile_pool(name="const", bufs=1) as cpool,
            tc.tile_pool(name="cbc", bufs=1) as cbc_pool,
            tc.tile_pool(name="xtr", bufs=1) as xtr_pool,
            tc.tile_pool(name="ch", bufs=1) as ch_pool,
            tc.tile_pool(name="wgu", bufs=2) as wgu_pool,
            tc.tile_pool(name="wd", bufs=3) as wd_pool,
            tc.tile_pool(name="act", bufs=3) as act_pool,
            tc.tile_pool(name="small", bufs=2) as sm_pool,
            tc.tile_pool(name="xtf", bufs=3) as xtf_pool,
            tc.tile_pool(name="ob", bufs=4) as ob_pool,
            tc.tile_pool(name="psr", bufs=2, space="PSUM") as psr_pool,
            tc.tile_pool(name="psa", bufs=2, space="PSUM") as psa_pool,
            tc.tile_pool(name="psb", bufs=2, space="PSUM") as psb_pool,
            tc.tile_pool(name="dram", bufs=1, space="DRAM") as dram_pool,
        ):
            # ---- constants ----
            ident = cpool.tile([128, 128], F32)
            make_identity(nc, ident[:])
            gwT_sb = cpool.tile([128, NHC, E], F32)
            nc.sync.dma_start(
                gwT_sb[:], gwT_d[:].rearrange("(c p) e -> p c e", p=128))
            esel_sb = cpool.tile([E, EPC * 128], F32)
            nc.sync.dma_start(esel_sb[:], esel_d[:])

            # ---- x^T bf16, resident: [128, 16 h-chunks, 1024] ----
            xTr = xtr_pool.tile([128, NHC, T], BF16, tag="xTr")
            for hc in range(NHC):
                nc.sync.dma_start(xTr[:, hc, :],
                                  xT_d[hc * 128:(hc + 1) * 128, :])

            # ---- router (fp32): logits -> top-6 combine weights ----
            combT = cpool.tile([E, T], F32)
            for tt in range(T // 128):
                psl = psr_pool.tile([128, E], F32, tag="psr")
                for hc in range(NHC):
                    xtf = xtf_pool.tile([128, 128], F32, tag="xtf")
                    nc.sync.dma_start(
                        xtf[:],
                        xT32_d[hc * 128:(hc + 1) * 128,
                               tt * 128:(tt + 1) * 128])
                    nc.tensor.matmul(psl[:], xtf[:], gwT_sb[:, hc, :],
                                     start=(hc == 0), stop=(hc == NHC - 1))
                mx = sm_pool.tile([128, 1], F32, tag="mx")
                nc.vector.reduce_max(mx[:], psl[:], axis=AX.X)
                ee = sm_pool.tile([128, E], F32, tag="ee")
                nc.vector.tensor_scalar(ee[:], psl[:], mx[:], None,
                                        op0=ALU.subtract)
                nc.scalar.activation(ee[:], ee[:], ACTF.Exp)
                top8 = sm_pool.tile([128, 8], F32, tag="top8")
                nc.vector.max(out=top8[:], in_=ee[:])
                mask = sm_pool.tile([128, E], F32, tag="mask")
                nc.vector.tensor_scalar(mask[:], ee[:], top8[:, K - 1:K],
                                        None, op0=ALU.is_ge)
                s6 = sm_pool.tile([128, 1], F32, tag="s6")
                nc.vector.reduce_sum(s6[:], top8[:, 0:K], axis=AX.X)
                r6 = sm_pool.tile([128, 1], F32, tag="r6")
                nc.vector.reciprocal(r6[:], s6[:])
                num = sm_pool.tile([128, E], F32, tag="num")
                nc.vector.tensor_mul(num[:], ee[:], mask[:])
                comb = sm_pool.tile([128, E], F32, tag="comb")
                nc.vector.tensor_scalar(comb[:], num[:], r6[:], None,
                                        op0=ALU.mult)
                pst = psr_pool.tile([E, 128], F32, tag="psr")
                nc.tensor.transpose(pst[:], comb[:], ident[:])
                nc.scalar.copy(combT[:, tt * 128:(tt + 1) * 128], pst[:])

            # broadcast this core's two experts' comb rows to all partitions
            cbc = cbc_pool.tile([128, EPC, T], F32, tag="cbc")
            for j in range(EPC):
                for tb in range(NTB):
                    pscb = psr_pool.tile([128, TB], F32, tag="psr")
                    nc.tensor.matmul(pscb[:], esel_sb[:, j * 128:(j + 1) * 128],
                                     combT[:, tb * TB:(tb + 1) * TB],
                                     start=True, stop=True)
                    nc.vector.tensor_copy(cbc[:, j, tb * TB:(tb + 1) * TB],
                                          pscb[:])

            # ---- stage A: SwiGLU activations (bf16), comb pre-scaled ----
            ch_tiles = {}

            def stage_a(gT_ap, uT_ap, widths, ch_tag, comb_j):
                ch_tiles[ch_tag] = []
                for li, w in enumerate(widths):
                    i0 = sum(widths[:li])
                    wgc = wgu_pool.tile([128, NHC, 256], BF16, tag="wg")
                    wuc = wgu_pool.tile([128, NHC, 256], BF16, tag="wu")
                    nc.sync.dma_start(
                        wgc[:, :, :w],
                        gT_ap[:, i0:i0 + w].rearrange("(c p) i -> p c i", p=128))
                    nc.sync.dma_start(
                        wuc[:, :, :w],
                        uT_ap[:, i0:i0 + w].rearrange("(c p) i -> p c i", p=128))
                    for o in range(0, w, 128):
                        m = min(128, w - o)
                        ch = ch_pool.tile([128, T], BF16,
                                          tag=f"{ch_tag}_{i0 + o}")
                        ch_tiles[ch_tag].append((ch, m))
                        for tb in range(NTB):
                            t_ = slice(tb * TB, (tb + 1) * TB)
                            psg = psa_pool.tile([128, TB], F32, tag="psg")
                            psu = psa_pool.tile([128, TB], F32, tag="psu")
                            for hc in range(NHC):
                                nc.tensor.matmul(
                                    psg[:m], wgc[:, hc, o:o + m], xTr[:, hc, t_],
                                    start=(hc == 0), stop=(hc == NHC - 1))
                            for hc in range(NHC):
                                nc.tensor.matmul(
                                    psu[:m], wuc[:, hc, o:o + m], xTr[:, hc, t_],
                                    start=(hc == 0), stop=(hc == NHC - 1))
                            sg = act_pool.tile([128, TB], F32, tag="sg")
                            nc.scalar.activation(sg[:m], psg[:m], ACTF.Silu)
                            if comb_j is None:
                                nc.vector.tensor_mul(ch[:m, t_], sg[:m], psu[:m])
                            else:
                                tmp = act_pool.tile([128, TB], F32, tag="tmp")
                                nc.vector.tensor_mul(tmp[:m], sg[:m], psu[:m])
                                nc.vector.tensor_mul(ch[:m, t_], tmp[:m],
                                                     cbc[:, comb_j, t_])

            for j in range(EPC):
                stage_a(wgT_d[j], wuT_d[j], RT_I, f"ch{j}", j)
            stage_a(swgT_d[:], swuT_d[:], SH_I, "chs", None)

            # ---- stage B: fused down-projection ----
            # two RS collectives split along hidden dim for compute overlap
            ccin = [dram_pool.tile([T, H // 2], F32, name=f"ccin{v}")
                    for v in (0, 1)]
            ccout = [dram_pool.tile([T // NC, H // 2], F32, name=f"ccout{v}")
                     for v in (0, 1)]

            rt_mm = _mm_tiles(RT_I)    # 11 matmul tiles per routed expert
            sh_mm = _mm_tiles(SH_I)    # 3 for the shared slice
            n_acc = EPC * len(rt_mm) + len(sh_mm)

            for hb in range(NHB):
                h0 = hb * HB
                wds = []
                for j in range(EPC):
                    wd = wd_pool.tile([128, len(rt_mm), HB], BF16, tag=f"wd{j}")
                    nc.sync.dma_start(
                        wd[:],
                        wdT_d[j][:, h0:h0 + HB].rearrange(
                            "(c p) h -> p c h", p=128))
                    wds.append(wd)
                wsd = wd_pool.tile([128, len(sh_mm), HB], BF16, tag="wds")
                nc.sync.dma_start(
                    wsd[:, 0:2, :],
                    swdT_d[0:256, h0:h0 + HB].rearrange("(c p) h -> p c h", p=128))
                nc.sync.dma_start(wsd[:96, 2, :], swdT_d[256:352, h0:h0 + HB])

                for tt in range(T // 128):
                    ts_ = slice(tt * 128, (tt + 1) * 128)
                    ps = psb_pool.tile([128, HB], F32, tag="psb")
                    k = 0
                    for j in range(EPC):
                        for it, (ch, m) in enumerate(ch_tiles[f"ch{j}"]):
                            nc.tensor.matmul(ps[:], ch[:m, ts_], wds[j][:m, it, :],
                                             start=(k == 0),
                                             stop=(k == n_acc - 1))
                            k += 1
                    for it, (ch, m) in enumerate(ch_tiles["chs"]):
                        nc.tensor.matmul(ps[:], ch[:m, ts_], wsd[:m, it, :],
                                         start=(k == 0), stop=(k == n_acc - 1))
                        k += 1
                    ob = ob_pool.tile([128, HB], F32, tag="ob")
                    nc.scalar.copy(ob[:], ps[:])
                    v = hb // (NHB // 2)
                    nc.sync.dma_start(
                        ccin[v][ts_, h0 - v * (H // 2):h0 - v * (H // 2) + HB],
                        ob[:])

            rows = T // NC
            for v in (0, 1):
                nc.gpsimd.collective_compute(
                    "ReduceScatter",
                    ALU.add,
                    replica_groups=[list(range(NC))],
                    ins=[ccin[v][:].opt()],
                    outs=[ccout[v][:].opt()],
                )
                nc.sync.dma_start(out_d[:, v * (H // 2):(v + 1) * (H // 2)],
                                  ccout[v][:])

    nc.compile()
    nc.m = get_hw_module(nc.m)
    return nc


_PROGRAM = None


def _get_program():
    global _PROGRAM
    if _PROGRAM is None:
        _PROGRAM = _build_program()
    return _PROGRAM


def _prep_in_maps(x, gate_w, w_gate, w_up, w_down, sw_gate, sw_up, sw_down):
    f = np.float32
    bf = ml_dtypes.bfloat16
    xT32 = np.ascontiguousarray(np.asarray(x, f).T)                # [H, T]
    xT = xT32.astype(bf)
    gwT = np.ascontiguousarray(np.asarray(gate_w, f).T)            # [H, E]
    wgT = np.ascontiguousarray(
        np.asarray(w_gate, f).transpose(0, 2, 1)).astype(bf)
    wuT = np.ascontiguousarray(
        np.asarray(w_up, f).transpose(0, 2, 1)).astype(bf)
    wdT = np.ascontiguousarray(
        np.asarray(w_down, f).transpose(0, 2, 1)).astype(bf)
    swgT = np.ascontiguousarray(np.asarray(sw_gate, f).T).astype(bf)
    swuT = np.ascontiguousarray(np.asarray(sw_up, f).T).astype(bf)
    swdT = np.ascontiguousarray(np.asarray(sw_down, f).T).astype(bf)

    in_maps = []
    for r in range(NC):
        esel = np.zeros((E, EPC * 128), f)
        for j in range(EPC):
            esel[EPC * r + j, j * 128:(j + 1) * 128] = 1.0
        in_maps.append({
            "xT32": xT32,
            "xT": xT,
            "gwT": gwT,
            "wgT": np.ascontiguousarray(wgT[EPC * r:EPC * (r + 1)]),
            "wuT": np.ascontiguousarray(wuT[EPC * r:EPC * (r + 1)]),
            "wdT": np.ascontiguousarray(wdT[EPC * r:EPC * (r + 1)]),
            "swgT": np.ascontiguousarray(swgT[:, SIL * r:SIL * (r + 1)]),
            "swuT": np.ascontiguousarray(swuT[:, SIL * r:SIL * (r + 1)]),
            "swdT": np.ascontiguousarray(swdT[SIL * r:SIL * (r + 1), :]),
            "esel": esel,
        })
    return in_maps


def kernel(x, gate_w, w_gate, w_up, w_down, sw_gate, sw_up, sw_down,
           _trace=False):
    nc = _get_program()
    in_maps = _prep_in_maps(x, gate_w, w_gate, w_up, w_down,
                            sw_gate, sw_up, sw_down)
    res = bass_utils.run_bass_kernel_spmd(
        nc, in_maps, core_ids=list(range(NC)), trace=_trace)

    out = np.empty((T, H), np.float32)
    rows = T // NC
    for r in range(NC):
        out[rows * r:rows * (r + 1)] = res.results[r]["out"]
    if _trace:
        kernel._last_results = res
    return out


# revision 4
# speedup vs baseline: 1.1484x; 1.1081x over previous
"""DeepseekV2 MoE layer on 8 Trainium2 NeuronCores (Bass/Tile).

Strategy (expert-parallel, per sharding hint):
  - 16 routed experts sharded 2-per-core; shared-expert intermediate dim
    (2816) sharded 8-way (tensor-parallel Column/RowParallel style).
  - Router (gate) replicated: each core computes softmax-free top-6 combine
    weights for all 1024 tokens in fp32 (exact expert selection).
  - Expert MLPs run dense over all tokens in bf16 (f32 PSUM accumulate).
    Combine weights are pre-multiplied into the SwiGLU activations, so each
    core's two routed experts AND its shared-expert slice accumulate into a
    single PSUM group in the down-projection.
  - Per-core partial [1024, 2048] outputs are summed with two
    ReduceScatter collectives (split along hidden dim so the first
    overlaps the second half of the down-projection); host reassembles.

Weights are pre-transposed (contraction-major) and pre-cast to bf16
host-side: TensorE contracts along the SBUF partition dim, transposed-AP
DMA is ~19x slower, and bf16 halves HBM traffic + enables fast weight load.
"""

import numpy as np
import ml_dtypes

import concourse.bass as bass
import concourse.mybir as mybir
import concourse.tile as tile
from concourse import bacc
from concourse import bass_utils
from concourse.bass_interp import get_hw_module
from concourse.masks import make_identity

F32 = mybir.dt.float32
BF16 = mybir.dt.bfloat16
AX = mybir.AxisListType
ALU = mybir.AluOpType
ACTF = mybir.ActivationFunctionType

T = 1024      # tokens
H = 2048      # hidden
I = 1408      # moe intermediate
E = 16        # routed experts
K = 6         # experts per token
SI = 2816     # shared intermediate
NC = 8        # cores
EPC = E // NC            # experts per core (2)
SIL = SI // NC           # shared intermediate per core (352)
NHC = H // 128           # h chunks (16)
TB = 512                 # stage-A token block (psum free dim)
NTB = T // TB            # 2
RT_I = [256] * 5 + [128]   # routed i-tile loads (512B dma runs)
SH_I = [256, 96]           # shared i-tile loads
HB = 256                 # stage-B h block
NHB = H // HB            # 8 h blocks


def _mm_tiles(widths):
    """(load_idx, offset_in_load, global_i0, m_width) per 128-wide matmul."""
    out = []
    g0 = 0
    for li, w in enumerate(widths):
        o = 0
        while o < w:
            m = min(128, w - o)
            out.append((li, o, g0 + o, m))
            o += m
    return out


def _build_program():
    nc = bacc.Bacc("TRN2", target_bir_lowering=False, debug=False,
                   enable_asserts=False, num_devices=NC)

    xT32_d = nc.dram_tensor("xT32", [H, T], F32, kind="ExternalInput")
    xT_d = nc.dram_tensor("xT", [H, T], BF16, kind="ExternalInput")
    gwT_d = nc.dram_tensor("gwT", [H, E], F32, kind="ExternalInput")
    wgT_d = nc.dram_tensor("wgT", [EPC, H, I], BF16, kind="ExternalInput")
    wuT_d = nc.dram_tensor("wuT", [EPC, H, I], BF16, kind="ExternalInput")
    wdT_d = nc.dram_tensor("wdT", [EPC, I, H], BF16, kind="ExternalInput")
    swgT_d = nc.dram_tensor("swgT", [H, SIL], BF16, kind="ExternalInput")
    swuT_d = nc.dram_tensor("swuT", [H, SIL], BF16, kind="ExternalInput")
    swdT_d = nc.dram_tensor("swdT", [SIL, H], BF16, kind="ExternalInput")
    esel_d = nc.dram_tensor("esel", [E, EPC * 128], F32, kind="ExternalInput")
    out_d = nc.dram_tensor("out", [T // NC, H], F32, kind="ExternalOutput")

    with tile.TileContext(nc) as tc:
        with (
            tc.tile_pool(name="const", bufs=1) as cpool,
            tc.tile_pool(name="cbc", bufs=1) as cbc_pool,
            tc.tile_pool(name="xtr", bufs=1) as xtr_pool,
            tc.tile_pool(name="ch", bufs=1) as ch_pool,
            tc.tile_pool(name="wgu", bufs=2) as wgu_pool,
            tc.tile_pool(name="wd", bufs=3) as wd_pool,
            tc.tile_pool(name="act", bufs=3) as act_pool,
            tc.tile_pool(name="small", bufs=2) as sm_pool,
            tc.tile_pool(name="xtf", bufs=6) as xtf_pool,
            tc.tile_pool(name="ob", bufs=4) as ob_pool,
            tc.tile_pool(name="psr", bufs=2, space="PSUM") as psr_pool,
            tc.tile_pool(name="psa", bufs=2, space="PSUM") as psa_pool,
            tc.tile_pool(name="psb", bufs=2, space="PSUM") as psb_pool,
            tc.tile_pool(name="dram", bufs=1, space="DRAM") as dram_pool,
        ):
            # ---- constants ----
            ident = cpool.tile([128, 128], F32)
            make_identity(nc, ident[:])
            gwT_sb = cpool.tile([128, NHC, E], F32)
            nc.sync.dma_start(
                gwT_sb[:], gwT_d[:].rearrange("(c p) e -> p c e", p=128))
            esel_sb = cpool.tile([E, EPC * 128], F32)
            nc.sync.dma_start(esel_sb[:], esel_d[:])

            # ---- x^T bf16, resident: [128, 16 h-chunks, 1024] ----
            xTr = xtr_pool.tile([128, NHC, T], BF16, tag="xTr")
            for hc in range(NHC):
                nc.sync.dma_start(xTr[:, hc, :],
                                  xT_d[hc * 128:(hc + 1) * 128, :])

            # ---- router (fp32): logits -> top-6 combine weights ----
            combT = cpool.tile([E, T], F32)
            for tt in range(T // 128):
                psl = psr_pool.tile([128, E], F32, tag="psr")
                for hc in range(NHC):
                    xtf = xtf_pool.tile([128, 128], F32, tag="xtf")
                    nc.sync.dma_start(
                        xtf[:],
                        xT32_d[hc * 128:(hc + 1) * 128,
                               tt * 128:(tt + 1) * 128])
                    nc.tensor.matmul(psl[:], xtf[:], gwT_sb[:, hc, :],
                                     start=(hc == 0), stop=(hc == NHC - 1))
                mx = sm_pool.tile([128, 1], F32, tag="mx")
                nc.vector.reduce_max(mx[:], psl[:], axis=AX.X)
                ee = sm_pool.tile([128, E], F32, tag="ee")
                nc.vector.tensor_scalar(ee[:], psl[:], mx[:], None,
                                        op0=ALU.subtract)
                nc.scalar.activation(ee[:], ee[:], ACTF.Exp)
                top8 = sm_pool.tile([128, 8], F32, tag="top8")
                nc.vector.max(out=top8[:], in_=ee[:])
                mask = sm_pool.tile([128, E], F32, tag="mask")
                nc.vector.tensor_scalar(mask[:], ee[:], top8[:, K - 1:K],
                                        None, op0=ALU.is_ge)
                s6 = sm_pool.tile([128, 1], F32, tag="s6")
                nc.vector.reduce_sum(s6[:], top8[:, 0:K], axis=AX.X)
                r6 = sm_pool.tile([128, 1], F32, tag="r6")
                nc.vector.reciprocal(r6[:], s6[:])
                num = sm_pool.tile([128, E], F32, tag="num")
                nc.vector.tensor_mul(num[:], ee[:], mask[:])
                comb = sm_pool.tile([128, E], F32, tag="comb")
                nc.vector.tensor_scalar(comb[:], num[:], r6[:], None,
                                        op0=ALU.mult)
                pst = psr_pool.tile([E, 128], F32, tag="psr")
                nc.tensor.transpose(pst[:], comb[:], ident[:])
                nc.scalar.copy(combT[:, tt * 128:(tt + 1) * 128], pst[:])

            # broadcast this core's two experts' comb rows to all partitions
            cbc = cbc_pool.tile([128, EPC, T], F32, tag="cbc")
            for j in range(EPC):
                for tb in range(NTB):
                    pscb = psr_pool.tile([128, TB], F32, tag="psr")
                    nc.tensor.matmul(pscb[:], esel_sb[:, j * 128:(j + 1) * 128],
                                     combT[:, tb * TB:(tb + 1) * TB],
                                     start=True, stop=True)
                    nc.vector.tensor_copy(cbc[:, j, tb * TB:(tb + 1) * TB],
                                          pscb[:])

            # ---- stage A: SwiGLU activations (bf16), comb pre-scaled ----
            ch_tiles = {}

            def stage_a(gT_ap, uT_ap, widths, ch_tag, comb_j):
                ch_tiles[ch_tag] = []
                for li, w in enumerate(widths):
                    i0 = sum(widths[:li])
                    wgc = wgu_pool.tile([128, NHC, 256], BF16, tag="wg")
                    wuc = wgu_pool.tile([128, NHC, 256], BF16, tag="wu")
                    nc.sync.dma_start(
                        wgc[:, :, :w],
                        gT_ap[:, i0:i0 + w].rearrange("(c p) i -> p c i", p=128))
                    nc.sync.dma_start(
                        wuc[:, :, :w],
                        uT_ap[:, i0:i0 + w].rearrange("(c p) i -> p c i", p=128))
                    for o in range(0, w, 128):
                        m = min(128, w - o)
                        ch = ch_pool.tile([128, T], BF16,
                                          tag=f"{ch_tag}_{i0 + o}")
                        ch_tiles[ch_tag].append((ch, m))
                        for tb in range(NTB):
                            t_ = slice(tb * TB, (tb + 1) * TB)
                            psg = psa_pool.tile([128, TB], F32, tag="psg")
                            psu = psa_pool.tile([128, TB], F32, tag="psu")
                            for hc in range(NHC):
                                nc.tensor.matmul(
                                    psg[:m], wgc[:, hc, o:o + m], xTr[:, hc, t_],
                                    start=(hc == 0), stop=(hc == NHC - 1))
                            for hc in range(NHC):
                                nc.tensor.matmul(
                                    psu[:m], wuc[:, hc, o:o + m], xTr[:, hc, t_],
                                    start=(hc == 0), stop=(hc == NHC - 1))
                            sg = act_pool.tile([128, TB], F32, tag="sg")
                            nc.scalar.activation(sg[:m], psg[:m], ACTF.Silu)
                            if comb_j is None:
                                nc.vector.tensor_mul(ch[:m, t_], sg[:m], psu[:m])
                            else:
                                tmp = act_pool.tile([128, TB], F32, tag="tmp")
                                nc.vector.tensor_mul(tmp[:m], sg[:m], psu[:m])
                                nc.vector.tensor_mul(ch[:m, t_], tmp[:m],
                                                     cbc[:, comb_j, t_])

            for j in range(EPC):
                stage_a(wgT_d[j], wuT_d[j], RT_I, f"ch{j}", j)
            stage_a(swgT_d[:], swuT_d[:], SH_I, "chs", None)

            # ---- stage B: fused down-projection ----
            # two RS collectives split along hidden dim for compute overlap
            NRS = 4
            HRS = H // NRS
            ccin = [dram_pool.tile([T, HRS], F32, name=f"ccin{v}")
                    for v in range(NRS)]
            ccout = [dram_pool.tile([T // NC, HRS], F32, name=f"ccout{v}")
                     for v in range(NRS)]

            rt_mm = _mm_tiles(RT_I)    # 11 matmul tiles per routed expert
            sh_mm = _mm_tiles(SH_I)    # 3 for the shared slice
            n_acc = EPC * len(rt_mm) + len(sh_mm)

            for hb in range(NHB):
                h0 = hb * HB
                wds = []
                for j in range(EPC):
                    wd = wd_pool.tile([128, len(rt_mm), HB], BF16, tag=f"wd{j}")
                    nc.sync.dma_start(
                        wd[:],
                        wdT_d[j][:, h0:h0 + HB].rearrange(
                            "(c p) h -> p c h", p=128))
                    wds.append(wd)
                wsd = wd_pool.tile([128, len(sh_mm), HB], BF16, tag="wds")
                nc.sync.dma_start(
                    wsd[:, 0:2, :],
                    swdT_d[0:256, h0:h0 + HB].rearrange("(c p) h -> p c h", p=128))
                nc.sync.dma_start(wsd[:96, 2, :], swdT_d[256:352, h0:h0 + HB])

                for tt in range(T // 128):
                    ts_ = slice(tt * 128, (tt + 1) * 128)
                    ps = psb_pool.tile([128, HB], F32, tag="psb")
                    k = 0
                    for j in range(EPC):
                        for it, (ch, m) in enumerate(ch_tiles[f"ch{j}"]):
                            nc.tensor.matmul(ps[:], ch[:m, ts_], wds[j][:m, it, :],
                                             start=(k == 0),
                                             stop=(k == n_acc - 1))
                            k += 1
                    for it, (ch, m) in enumerate(ch_tiles["chs"]):
                        nc.tensor.matmul(ps[:], ch[:m, ts_], wsd[:m, it, :],
                                         start=(k == 0), stop=(k == n_acc - 1))
                        k += 1
                    ob = ob_pool.tile([128, HB], F32, tag="ob")
                    nc.scalar.copy(ob[:], ps[:])
                    v = hb // (NHB // NRS)
                    nc.sync.dma_start(
                        ccin[v][ts_, h0 - v * HRS:h0 - v * HRS + HB], ob[:])
                if (hb + 1) % (NHB // NRS) == 0:
                    v = hb // (NHB // NRS)
                    nc.gpsimd.collective_compute(
                        "ReduceScatter",
                        ALU.add,
                        replica_groups=[list(range(NC))],
                        ins=[ccin[v][:].opt()],
                        outs=[ccout[v][:].opt()],
                    )
                    nc.sync.dma_start(out_d[:, v * HRS:(v + 1) * HRS],
                                      ccout[v][:])

    nc.compile()
    nc.m = get_hw_module(nc.m)
    return nc


_PROGRAM = None


def _get_program():
    global _PROGRAM
    if _PROGRAM is None:
        _PROGRAM = _build_program()
    return _PROGRAM


def _prep_in_maps(x, gate_w, w_gate, w_up, w_down, sw_gate, sw_up, sw_down):
    f = np.float32
    bf = ml_dtypes.bfloat16
    xT32 = np.ascontiguousarray(np.asarray(x, f).T)                # [H, T]
    xT = xT32.astype(bf)
    gwT = np.ascontiguousarray(np.asarray(gate_w, f).T)            # [H, E]
    wgT = np.ascontiguousarray(
        np.asarray(w_gate, f).transpose(0, 2, 1)).astype(bf)
    wuT = np.ascontiguousarray(
        np.asarray(w_up, f).transpose(0, 2, 1)).astype(bf)
    wdT = np.ascontiguousarray(
        np.asarray(w_down, f).transpose(0, 2, 1)).astype(bf)
    swgT = np.ascontiguousarray(np.asarray(sw_gate, f).T).astype(bf)
    swuT = np.ascontiguousarray(np.asarray(sw_up, f).T).astype(bf)
    swdT = np.ascontiguousarray(np.asarray(sw_down, f).T).astype(bf)

    in_maps = []
    for r in range(NC):
        esel = np.zeros((E, EPC * 128), f)
        for j in range(EPC):
            esel[EPC * r + j, j * 128:(j + 1) * 128] = 1.0
        in_maps.append({
            "xT32": xT32,
            "xT": xT,
            "gwT": gwT,
            "wgT": np.ascontiguousarray(wgT[EPC * r:EPC * (r + 1)]),
            "wuT": np.ascontiguousarray(wuT[EPC * r:EPC * (r + 1)]),
            "wdT": np.ascontiguousarray(wdT[EPC * r:EPC * (r + 1)]),
            "swgT": np.ascontiguousarray(swgT[:, SIL * r:SIL * (r + 1)]),
            "swuT": np.ascontiguousarray(swuT[:, SIL * r:SIL * (r + 1)]),
            "swdT": np.ascontiguousarray(swdT[SIL * r:SIL * (r + 1), :]),
            "esel": esel,
        })
    return in_maps


def kernel(x, gate_w, w_gate, w_up, w_down, sw_gate, sw_up, sw_down,
           _trace=False):
    nc = _get_program()
    in_maps = _prep_in_maps(x, gate_w, w_gate, w_up, w_down,
                            sw_gate, sw_up, sw_down)
    res = bass_utils.run_bass_kernel_spmd(
        nc, in_maps, core_ids=list(range(NC)), trace=_trace)

    out = np.empty((T, H), np.float32)
    rows = T // NC
    for r in range(NC):
        out[rows * r:rows * (r + 1)] = res.results[r]["out"]
    if _trace:
        kernel._last_results = res
    return out
